# revision 17
# baseline (speedup 1.0000x reference)
"""Distributed Bass attention kernel for 8 TRN2 NeuronCores.

Problem: full-dim attention (no head split), x:(2,4096,2048), 4x 2048^2 weights.

Since there is no head split, the weights compose:
    scores = X (Wq^T Wk) X^T = X M X^T
    out    = softmax(scores/sqrt(D)) X (Wo Wv)^T = P (X N^T) / l
so the q/k projections collapse into one projection by M = Wq^T Wk, and the
v/out projections collapse into one projection by N^T = Wv^T Wo^T.  M and N^T
are row-count independent, so their 2048^3 construction is sharded across all
8 cores and AllGather-ed with the fast intra-chip 8-core RDH algorithm
(~50us, vs ~200us for a 4-core ring AG).  Each core's shard is a [1024,512]
BLOCK (a-half x b-quarter), not a column slice: a block only needs 4MB+2MB
of the two source weights, so the weight streaming of the shard phase fits
the ~180GB/s effective HBM rate instead of stalling the PE (a column slice
needs a full 8MB weight).  Per-core matmul work: 34.4 -> 27.9 GMAC.

Sharding: batch+sequence parallel. Core c owns batch b=c//4 and query rows
[1024*(c%4), 1024*(c%4+1)).  The "keys" of the composed attention are the raw
inputs X, so no key gather is needed at all: the host hands every core its
whole batch's X^T (input upload is not part of the timed kernel), and the
score phase streams key chunks straight from that DRAM input.  The only
collectives are AG(N^T), AG(M) (8-core RDH) and AG(Vt) (4-core ring, the Vt
shards are computed on-device).  The CC queue is serialized, so they are
chained explicitly in that order; AG(Vt) is additionally held back until the
4th key-chunk DMA has issued so the kbuf prefetch streams at full HBM rate
before the ring's SDMA traffic contends, and still lands long before ctx.

Phase order is chosen so each AllGather gets a wide landing window:
  1. N^T-shard, AG(N^T); M-shard, AG(M)       (2x 256 N=256 MMs, ~68us)
  2. Vt = X N^T -> AG(Vt)                      (512 N=512 MMs, 134us)
  3. qt = (X M)^T                              (512 N=512 MMs, 134us)
  4. scores^T + exp streamed over key chunks; rowsum ones-matmuls interleaved
     (pipelined one slice behind exp)          (1024+64 MMs, ~286us)
  5. out^T[f,i] = sum_j Vt[j,f]^T P^T[j,i] / l (1024 MMs, 269us), 1/l applied
     by DVE during PSUM evacuation; stored as [D,ROWS], host transposes.

DMA discipline: weight tiles alternate the sync/scalar HWDGE rings (the
N=256 shard phases stream lhsT at ~250GB/s, near the HBM roofline); M/N
staging is partition-major so each store is one descriptor on the gpsimd
queue (where it precedes its AllGather anyway); outputs go on scalar, vcol
streams on sync.  All TensorE math bf16 with fp32 PSUM accumulation.
"""

import numpy as np
import ml_dtypes

BF16 = ml_dtypes.bfloat16

D = 2048          # model dim
S = 4096          # sequence length per batch
BATCH = 2
NCORES = 8
GROUP = 4         # cores per batch
ROWS = S // GROUP  # query rows per core = 1024
P = 128           # partitions
DT = D // P       # 16 d-tiles
IT = ROWS // P    # 8 i-tiles per core
JT = S // P       # 32 j-tiles (full seq)
NCH = S // 512    # 8 key chunks
CS = D // NCORES  # 256 (M/N^T shard columns when column-sharded)
HD = DT // 2      # 8: a/d tiles per block shard
SCALE = 1.0 / float(np.sqrt(D))

_CACHE = {}


def _build():
    from concourse import bacc, mybir, tile
    from concourse.bass import _add_dep_helper

    f32 = mybir.dt.float32
    bf16 = mybir.dt.bfloat16

    nc = bacc.Bacc("TRN2", target_bir_lowering=False, debug=False,
                   num_devices=NCORES)

    # host-pre-tiled inputs: every load is a contiguous block
    xt_d = nc.dram_tensor("xt", [P, DT * ROWS], bf16, kind="ExternalInput")
    xk_d = nc.dram_tensor("xk", [P, DT * S], bf16, kind="ExternalInput")
    # wqh[atl,p,et*128+j] = wq[et*128+p, 1024*(c//4)+atl*128+j]
    wqh_d = nc.dram_tensor("wqh", [HD, P, DT * P], bf16, kind="ExternalInput")
    # wksl[p, et*512+j] = wk[et*128+p, 512*(c%4)+j]
    wksl_d = nc.dram_tensor("wksl", [P, DT * 512], bf16, kind="ExternalInput")
    # wvh[dtl,p,et*128+j] = wv[et*128+p, 1024*(c//4)+dtl*128+j]
    wvh_d = nc.dram_tensor("wvh", [HD, P, DT * P], bf16, kind="ExternalInput")
    # wosl[p, et*512+j] = wo[512*(c%4)+j, et*128+p]
    wosl_d = nc.dram_tensor("wosl", [P, DT * 512], bf16, kind="ExternalInput")
    out_d = nc.dram_tensor("out", [D, ROWS], f32, kind="ExternalOutput")

    RG8 = [list(range(NCORES))]
    RG4 = [[0, 1, 2, 3], [4, 5, 6, 7]]
    RGP = [[0, 4], [1, 5], [2, 6], [3, 7]]  # partner pairs (same f-qtr)

    def all_gather(src, dst, rg, after=None):
        cc = nc.gpsimd.collective_compute(
            "AllGather", mybir.AluOpType.bypass, replica_groups=rg,
            ins=[src.opt()], outs=[dst.opt()])
        if after is not None:
            _add_dep_helper(cc.ins, after.ins, sync=True,
                            reason="serialize CC queue order")
        return cc

    with tile.TileContext(nc) as tc:
        with (
            tc.tile_pool(name="dram", bufs=1, space="DRAM") as dram,
            tc.tile_pool(name="persist", bufs=1) as persist,
            tc.tile_pool(name="psum", bufs=2, space="PSUM") as psum,
        ):
            # partition-major M/N staging: single-descriptor stores +
            # loads.  mg[4*ah+bq][p,atl,b'] = M[1024*ah+128*atl+p, 512*bq+b']
            m_b = dram.tile([P, HD, 512], bf16)
            n_b = dram.tile([P, HD, 512], bf16)
            vq_b = dram.tile([S, 512], bf16)
            mg = dram.tile([NCORES, P, HD, 512], bf16)
            # pairwise d-half exchange: ngx[dh] = N^T[d-half dh, my f-qtr]
            ngx = dram.tile([2, P, HD, 512], bf16)
            vgq = dram.tile([GROUP, S, 512], bf16)

            linv_bc = persist.tile([P, ROWS], f32)  # 1/l bcast on partitions
            ones = persist.tile([P, P], bf16)
            # memset now: the gpsimd FIFO later holds the collectives, and
            # anything emitted after them waits for AG(Vt) to finish
            nc.gpsimd.memset(ones[:], 1.0)

            with tc.tile_pool(name="qtpool", bufs=1) as qtpool:
                qt_s = qtpool.tile([P, DT, ROWS], bf16)  # (X M)^T [e, i]

                # -------- Phase 1: N^T / M shards + projections --------
                with tc.tile_pool(name="proj", bufs=2) as proj:
                    # warm both HWDGE rings
                    warm = proj.tile([P, 16], bf16, bufs=1)
                    nc.sync.dma_start(out=warm[0:1, :], in_=xt_d[0:1, 0:16])
                    nc.scalar.dma_start(out=warm[1:2, :], in_=xt_d[1:2, 0:16])

                    wo_s = proj.tile([P, DT * 512], bf16, bufs=1)
                    for h in range(2):
                        eng = nc.scalar if h == 0 else nc.sync
                        eng.dma_start(out=wo_s[:, h * 4096:(h + 1) * 4096],
                                      in_=wosl_d[:, h * 4096:(h + 1) * 4096])

                    def mn_shard(w_d, rhs_s, dst_b):
                        # dst[p,atl,b'] = sum_et w[et, a-tile]^T rhs[et, b']
                        st = proj.tile([P, HD, 512], bf16, tag="mn_st",
                                       bufs=1)
                        # emit ALL paired 1MB weight loads first: the HWDGE
                        # queues are strictly in-order, so the staging
                        # stores (emitted later) never block a prefetch
                        wcols = []
                        for ap in range(HD // 2):
                            wcol = proj.tile([P, 2, DT * P], bf16,
                                             tag="wcol", bufs=4,
                                             name=f"wcol{ap}")
                            eng = nc.sync if ap % 2 == 0 else nc.scalar
                            eng.dma_start(
                                out=wcol[:],
                                in_=w_d[2 * ap:2 * ap + 2].rearrange(
                                    "a p e -> p a e"))
                            wcols.append(wcol)
                        for at in range(HD):
                            wcol = wcols[at // 2]
                            ps = psum.tile([P, 512], f32, tag="acc")
                            for et in range(DT):
                                nc.tensor.matmul(
                                    ps[:],
                                    wcol[:, at % 2, et * P:(et + 1) * P],
                                    rhs_s[:, et * 512:(et + 1) * 512],
                                    start=(et == 0),
                                    stop=(et == DT - 1))
                            nc.vector.tensor_copy(st[:, at, :], ps[:])
                            # incremental staging stores, alternating rings;
                            # the shard's AllGather trigger only waits for
                            # the last one (~0.5us after the last evac)
                            eng = nc.sync if at % 2 == 0 else nc.scalar
                            eng.dma_start(out=dst_b[:, at, :],
                                          in_=st[:, at, :])

                    mn_shard(wvh_d, wo_s, n_b)
                    agnx = all_gather(n_b, ngx, RGP)
                    wk_s = proj.tile([P, DT * 512], bf16, bufs=1)
                    for h in range(2):
                        eng = nc.scalar if h == 0 else nc.sync
                        eng.dma_start(out=wk_s[:, h * 4096:(h + 1) * 4096],
                                      in_=wksl_d[:, h * 4096:(h + 1) * 4096])
                    mn_shard(wqh_d, wk_s, m_b)
                    agm = all_gather(m_b, mg, RG8, after=agnx)

                    # Vq = X_batch N^T[:, my f-quarter]: all 4096 rows,
                    # 512 f-columns.  Needs only the two pairwise-exchanged
                    # N^T blocks, not the full 8-core AllGather; the keys
                    # input xk provides X^T for the whole batch.
                    ncolq = proj.tile([P, DT, 512], bf16, bufs=1)
                    for dh in range(2):
                        eng = nc.sync if dh == 0 else nc.scalar
                        eng.dma_start(
                            out=ncolq[:, dh * HD:(dh + 1) * HD, :],
                            in_=ngx[dh])
                    xk_v = xk_d[:].rearrange("p (t j) -> p t j", t=DT)
                    for jb in range(NCH):  # 8 j-blocks of 512 rows
                        xkc = proj.tile([P, DT, 512], bf16, tag="bigcol",
                                        bufs=2, name=f"xkc{jb}")
                        for h in range(2):
                            eng = nc.sync if h == 0 else nc.scalar
                            eng.dma_start(
                                out=xkc[:, :, h * 256:(h + 1) * 256],
                                in_=xk_v[:, :, jb * 512 + h * 256:
                                         jb * 512 + (h + 1) * 256])
                        v_st = proj.tile([P, 4, 512], bf16, tag="v_st",
                                         bufs=2)
                        for jtl in range(4):
                            ps = psum.tile([P, 512], f32, tag="acc")
                            for dt_i in range(DT):
                                nc.tensor.matmul(
                                    ps[:],
                                    xkc[:, dt_i, jtl * P:(jtl + 1) * P],
                                    ncolq[:, dt_i, :],
                                    start=(dt_i == 0),
                                    stop=(dt_i == DT - 1))
                            nc.vector.tensor_copy(v_st[:, jtl, :], ps[:])
                        eng = nc.sync if jb % 2 == 0 else nc.scalar
                        eng.dma_start(
                            out=vq_b[jb * 512:(jb + 1) * 512, :]
                            .rearrange("(t p) f -> p t f", p=P),
                            in_=v_st[:])
                    agv = all_gather(vq_b, vgq, RG4, after=agm)

                    # x^T (own rows) into SBUF for the qt projection
                    xt_s = proj.tile([P, DT, ROWS], bf16, bufs=1)
                    xt_v = xt_d[:].rearrange("p (t i) -> p t i", t=DT)
                    for c in range(2):
                        eng = nc.sync if c == 0 else nc.scalar
                        eng.dma_start(
                            out=xt_s[:, :, c * 512:(c + 1) * 512],
                            in_=xt_v[:, :, c * 512:(c + 1) * 512])

                    # qt = (X M)^T; b-quarter bq needs the two a-half
                    # shards mg[bq] (a-tiles 0-7) and mg[4+bq] (8-15)
                    for bq in range(4):
                        mcol = proj.tile([P, DT, 512], bf16, tag="bigcol",
                                         bufs=2, name=f"mcol{bq}")
                        for ah in range(2):
                            eng = nc.sync if ah == 0 else nc.scalar
                            eng.dma_start(
                                out=mcol[:, ah * HD:(ah + 1) * HD, :],
                                in_=mg[4 * ah + bq])
                        for q in range(4):
                            for ic in range(2):
                                ps = psum.tile([P, 512], f32, tag="acc")
                                for at in range(DT):
                                    nc.tensor.matmul(
                                        ps[:],
                                        mcol[:, at, q * P:(q + 1) * P],
                                        xt_s[:, at, ic * 512:(ic + 1) * 512],
                                        start=(at == 0),
                                        stop=(at == DT - 1))
                                nc.vector.tensor_copy(
                                    qt_s[:, 4 * bq + q,
                                         ic * 512:(ic + 1) * 512], ps[:])

                # ------------- Phase 2: attention -------------
                with tc.tile_pool(name="attn", bufs=2) as attn:
                    pt_s = attn.tile([P, JT, IT * P], bf16, bufs=1)
                    lps = [psum.tile([P, 512], f32, tag="ctx",
                                     name=f"lps{ib}") for ib in range(2)]
                    # --- scores^T + exp + interleaved rowsums (pipelined by
                    #     one slice so the ones-matmul never waits on exp) ---
                    pend = []          # slices whose rowsum MM is not emitted
                    rcount = [0, 0]    # rowsum MMs emitted per ib

                    def emit_rowsum(jt, ib):
                        nc.tensor.matmul(
                            lps[ib][:], ones[:],
                            pt_s[:, jt, ib * 512:(ib + 1) * 512],
                            start=(rcount[ib] == 0),
                            stop=(rcount[ib] == JT - 1))
                        rcount[ib] += 1

                    xk_v = xk_d[:].rearrange("p (t j) -> p t j", t=DT)
                    for cidx in range(NCH):  # 8 chunks of 512 keys
                        kb = attn.tile([P, DT, 512], bf16, tag="kbuf",
                                       bufs=4)
                        for h in range(2):
                            eng = nc.sync if (cidx + h) % 2 == 0 else \
                                nc.scalar
                            kd = eng.dma_start(
                                out=kb[:, :, h * 256:(h + 1) * 256],
                                in_=xk_v[:, :, cidx * 512 + h * 256:
                                         cidx * 512 + (h + 1) * 256])
                        if cidx == 3:
                            # hold AG(Vt) until the kbuf stream is mostly
                            # issued (ring SDMA contention makes concurrent
                            # HWDGE loads crawl)
                            _add_dep_helper(
                                agv.ins, kd.ins, sync=True,
                                reason="delay AG(Vt) past kbuf prefetch")
                        for kl in range(4):
                            jt = 4 * cidx + kl
                            for ib in range(2):
                                sps = psum.tile([P, 512], f32,
                                                tag="scores", bufs=3)
                                for e in range(DT):
                                    nc.tensor.matmul(
                                        sps[:],
                                        kb[:, e, kl * P:(kl + 1) * P],
                                        qt_s[:, e, ib * 512:(ib + 1) * 512],
                                        start=(e == 0),
                                        stop=(e == DT - 1))
                                nc.scalar.activation(
                                    pt_s[:, jt, ib * 512:(ib + 1) * 512],
                                    sps[:],
                                    mybir.ActivationFunctionType.Exp,
                                    scale=SCALE)
                                pend.append((jt, ib))
                                if len(pend) > 1:
                                    emit_rowsum(*pend.pop(0))
                    for jt, ib in pend:
                        emit_rowsum(jt, ib)
                    for ib in range(2):
                        nc.vector.reciprocal(
                            linv_bc[:, ib * 512:(ib + 1) * 512], lps[ib][:])
                    # --- out^T[f, i] = sum_j Vt[j,f]^T P^T[j,i], scaled
                    #     by 1/l during PSUM evacuation; stored as [D,ROWS]
                    for fb in range(NCH):  # 8 blocks of 256 f-columns
                        qf, off = fb // 2, (fb % 2) * 256
                        vcol = attn.tile([P, NCH, 4, 256], bf16,
                                         tag="vcol", bufs=2)
                        for g in range(NCH):  # j-block [512g, 512g+512)
                            nc.sync.dma_start(
                                out=vcol[:, g, :, :],
                                in_=vgq[qf, g * 512:(g + 1) * 512,
                                        off:off + 256]
                                .rearrange("(t p) d -> p t d", p=P))
                        for ds in range(2):
                            ft = fb * 2 + ds
                            ot_st = attn.tile([P, ROWS], f32, tag="ot",
                                              bufs=2)
                            for ih in range(2):
                                cps = psum.tile([P, 512], f32, tag="ctx")
                                for jt in range(JT):
                                    nc.tensor.matmul(
                                        cps[:],
                                        vcol[:, jt // 4, jt % 4,
                                             ds * P:(ds + 1) * P],
                                        pt_s[:, jt, ih * 512:
                                             (ih + 1) * 512],
                                        start=(jt == 0),
                                        stop=(jt == JT - 1))
                                nc.vector.tensor_tensor(
                                    out=ot_st[:, ih * 512:(ih + 1) * 512],
                                    in0=cps[:],
                                    in1=linv_bc[:, ih * 512:(ih + 1) * 512],
                                    op=mybir.AluOpType.mult)
                            nc.scalar.dma_start(
                                out=out_d[ft * P:(ft + 1) * P, :],
                                in_=ot_st[:])

    nc.compile()
    return nc


def _get_nc():
    if "nc" not in _CACHE:
        _CACHE["nc"] = _build()
    return _CACHE["nc"]


def _tile_lhs(w):
    # [e, d] weight -> lhsT tiles [at, p, et*128]: out[at,p,et*128+j]
    # = w[et*128+p, at*128+j]
    w = np.asarray(w, np.float32)
    t = w.reshape(DT, P, DT, P).transpose(2, 1, 0, 3)  # [at, p, et, j]
    return np.ascontiguousarray(t.reshape(DT, P, DT * P)).astype(BF16)


def _tile_rhs_slice(wt, q):
    # wt: [e, cols] matrix; slice cols [512q, 512q+512) -> [p, et*512]
    sl = np.asarray(wt, np.float32)[:, q * 512:(q + 1) * 512]  # [e, 512]
    t = sl.reshape(DT, P, 512).transpose(1, 0, 2)              # [p, et, j]
    return np.ascontiguousarray(t.reshape(P, DT * 512)).astype(BF16)


def _tile_xt(xrows):
    # [rows, d] -> x^T tiled [p, dt, rows] flattened
    n = xrows.shape[0]
    xt = xrows.T.reshape(DT, P, n).transpose(1, 0, 2)
    return np.ascontiguousarray(xt.reshape(P, DT * n)).astype(BF16)


def _in_maps(x, wq, wk, wv, wo):
    wqh = _tile_lhs(wq)   # [16 at, P, DT*P]
    wvh = _tile_lhs(wv)
    wk = np.asarray(wk, np.float32)
    woT = np.ascontiguousarray(np.asarray(wo, np.float32).T)  # [e, f]
    x = np.asarray(x, np.float32)
    xk = [_tile_xt(x[b]) for b in range(BATCH)]  # full-batch keys, shared
    wks = [_tile_rhs_slice(wk, q) for q in range(4)]
    wos = [_tile_rhs_slice(woT, q) for q in range(4)]
    maps = []
    for c in range(NCORES):
        b, r = c // GROUP, c % GROUP
        ah, bq = c // 4, c % 4  # block shard: a/d-half x b/f-quarter
        maps.append({"xt": _tile_xt(x[b, r * ROWS:(r + 1) * ROWS, :]),
                     "xk": xk[b],
                     "wqh": np.ascontiguousarray(wqh[8 * ah:8 * ah + 8]),
                     "wksl": wks[bq],
                     "wvh": np.ascontiguousarray(wvh[8 * ah:8 * ah + 8]),
                     "wosl": wos[bq]})
    return maps


def run(x, wq, wk, wv, wo, trace=False, **trace_kwargs):
    from concourse.bass_utils import run_bass_kernel_spmd
    nc = _get_nc()
    res = run_bass_kernel_spmd(nc, _in_maps(x, wq, wk, wv, wo),
                               list(range(NCORES)), trace=trace,
                               **trace_kwargs)
    out = np.empty((BATCH, S, D), np.float32)
    for c in range(NCORES):
        b, r = c // GROUP, c % GROUP
        out[b, r * ROWS:(r + 1) * ROWS, :] = res.results[c]["out"].T
    return out, res


def kernel(x, wq, wk, wv, wo):
    out, _ = run(x, wq, wk, wv, wo)
    return out
